# revision 15
# baseline (speedup 1.0000x reference)
"""Trainium2 Bass kernel for nn_AD_F_56384330662393 (dense_cnn, 3-iter diffusion).

Data-parallel over batch: 32 images -> 8 cores x 4 images.

Per-core algorithm (all in bf16, PSUM fp32):
  x stored as 5 row-tiles [128 part, 516 cols] with 2 ghost rows/cols (zeros);
  tile T partition p <-> padded row 124T+p  (padded row = image row + 2).
  Per iteration t (3), per image (4), per tile T (5), per channel k (4):
    d1 = sum_{q=-2..2} bandmatmul_q(x-window shifted by q cols)   [PSUM fp32]
         + 2 tiny edge-correction matmuls (cols j=0, j=511)
    p  = Lrelu(0.5*d1, alpha=sqrt(a_t))      (ScalarE, PSUM->SBUF bf16)
  per (image, T):  S = sum_k p_k^2  (VectorE);  x -= S  (VectorE, in place)
  seam rows (shared between adjacent tiles) re-synced with 2 small DMAs.

The band matrices (lhsT) encode the exact masked-shift semantics of the
reference's extract_features + conv composition, including row-boundary
anomalies; the two tiny matmuls fix the column-boundary anomalies at q=0.
"""
import numpy as np

SHIFTS = [(-1, 0), (1, 0), (0, -1), (0, 1), (-1, -1), (-1, 1), (1, -1), (1, 1)]
H = W = 512
NK = 4            # conv output channels
NT = 3            # iterations
NIMG = 4          # images per core
NCORES = 8
TR = 124          # out rows per tile
NTILES = 5        # 5 tiles: 4*124 + 16 rows
PADW = 516        # 512 + 2 ghost cols each side
VAR_OF_TILE = [0, 1, 1, 1, 2]   # band variant per tile: top / mid / bottom


def _build_bands(Wt):
    """Wt: [4,8,3,3] float64. Returns:
    main [3var, 5q, 4k, 128, 128], corrL [3var, 4k, 128, 128], corrR same."""
    i_all = np.arange(H)
    B = np.zeros((NK, 5, 5, H))       # [k, q+2, p+2, i]
    CL = np.zeros((NK, 5, H))
    CR = np.zeros((NK, 5, H))
    for c, (di, dj) in enumerate(SHIFTS):
        for u in (-1, 0, 1):
            Rm = ((i_all + u >= 0) & (i_all + u < H)
                  & (i_all + u + di >= 0) & (i_all + u + di < H)).astype(np.float64)
            p = u + di
            for v in (-1, 0, 1):
                q = v + dj
                w = Wt[:, c, u + 1, v + 1]
                B[:, q + 2, p + 2, :] += w[:, None] * Rm[None, :]
                if q == 0 and (v, dj) == (-1, 1):
                    CL[:, p + 2, :] -= w[:, None] * Rm[None, :]
                if q == 0 and (v, dj) == (1, -1):
                    CR[:, p + 2, :] -= w[:, None] * Rm[None, :]
    tiles_of_var = [0, 1, 4]
    main = np.zeros((3, 5, NK, 128, 128))
    cl = np.zeros((3, NK, 128, 128))
    cr = np.zeros((3, NK, 128, 128))
    for vi, T in enumerate(tiles_of_var):
        for m in range(128):
            i = TR * T + m - 2
            if not (0 <= i < H):
                continue
            for p in (-2, -1, 0, 1, 2):
                r = m + p
                if not (0 <= r < 128):
                    continue
                main[vi, :, :, r, m] = B[:, :, p + 2, i].T
                cl[vi, :, r, m] = CL[:, p + 2, i]
                cr[vi, :, r, m] = CR[:, p + 2, i]
    return main, cl, cr


def _build_graph(Wf, bf, af):
    """Construct the Bass graph. Wf [3,4,8,3,3], bf [3,4], af [3] numpy."""
    from contextlib import ExitStack
    import concourse.bass as bass
    import concourse.tile as tile
    from concourse import mybir

    nc = bass.Bass()
    x_ext = nc.declare_dram_parameter("x", [NIMG, H, W], mybir.dt.float32,
                                      isOutput=False)
    # host passes these pre-transposed to [r, ...] so the DMA is contiguous
    bands_ext = nc.declare_dram_parameter(
        "bands", [128, NT, 3, 5, NK, 128], mybir.dt.float32, isOutput=False)
    cl_ext = nc.declare_dram_parameter(
        "corrl", [128, NT, 3, NK, 128], mybir.dt.float32, isOutput=False)
    cr_ext = nc.declare_dram_parameter(
        "corrr", [128, NT, 3, NK, 128], mybir.dt.float32, isOutput=False)
    out_ext = nc.declare_dram_parameter("out", [NIMG, H, W], mybir.dt.float32,
                                        isOutput=True)

    bf16 = mybir.dt.bfloat16
    f32 = mybir.dt.float32
    LR = mybir.ActivationFunctionType.Prelu

    with tile.TileContext(nc) as tc:
        with ExitStack() as ctx:
            persist = ctx.enter_context(tc.tile_pool(name="persist", bufs=1))
            ppool = ctx.enter_context(tc.tile_pool(name="p", bufs=3))
            spool = ctx.enter_context(tc.tile_pool(name="s", bufs=3))
            psum = ctx.enter_context(tc.tile_pool(name="ps", bufs=8, space="PSUM"))

            # persistent band weights in SBUF (bf16, cast during DMA)
            bands = persist.tile([128, NT, 3, 5, NK, 128], bf16, tag="bands")
            clt = persist.tile([128, NT, 3, NK, 128], bf16, tag="cl")
            crt = persist.tile([128, NT, 3, NK, 128], bf16, tag="cr")
            nc.gpsimd.dma_start(out=bands, in_=bands_ext[:, :, :, :, :, :])
            nc.gpsimd.dma_start(out=clt, in_=cl_ext[:, :, :, :, :])
            nc.gpsimd.dma_start(out=crt, in_=cr_ext[:, :, :, :, :])

            # persistent x buffers, one per image
            xbufs = []
            for im in range(NIMG):
                xb = persist.tile([128, NTILES, PADW], bf16, tag=f"xb{im}")
                nc.vector.memset(xb, 0)
                # tiles 0..3 full 124 rows; tile 4 only 16 rows
                nc.gpsimd.dma_start(
                    out=xb[2:126, 0:4, 2:514],
                    in_=x_ext[im, 0:496, :].rearrange("(tt p) c -> p tt c", p=124))
                nc.gpsimd.dma_start(
                    out=xb[2:18, 4, 2:514],
                    in_=x_ext[im, 496:512, :])
                # fill the seam-halo partitions for iteration 0
                nc.sync.dma_start(out=xb[0:2, 1:5, :], in_=xb[124:126, 0:4, :])
                nc.sync.dma_start(out=xb[126:128, 0:4, :], in_=xb[2:4, 1:5, :])
                xbufs.append(xb)

            for t in range(NT):
                alpha = float(np.sqrt(af[t]))
                for im in range(NIMG):
                    xb = xbufs[im]
                    for T in range(NTILES):
                        v = VAR_OF_TILE[T]
                        ptile = ppool.tile([128, NK, 512], bf16, tag="ptile")
                        for k in range(NK):
                            acc = psum.tile([128, 512], f32, tag="acc")
                            for qi, q in enumerate((-2, -1, 0, 1, 2)):
                                nc.tensor.matmul(
                                    acc, bands[:, t, v, qi, k, :],
                                    xb[:, T, 2 + q: 514 + q],
                                    start=(qi == 0), stop=False)
                            nc.tensor.matmul(acc[:, 0:1], clt[:, t, v, k, :],
                                             xb[:, T, 2:3],
                                             start=False, stop=False)
                            nc.tensor.matmul(acc[:, 511:512], crt[:, t, v, k, :],
                                             xb[:, T, 513:514],
                                             start=False, stop=True)
                            bias = 0.5 * float(bf[t, k])
                            nc.scalar.activation(out=ptile[:, k, :], in_=acc,
                                                 func=LR, bias=bias, scale=0.5,
                                                 alpha=alpha)
                        # S = sum_k p_k^2 ; x -= S
                        pflat = ptile.rearrange("r k c -> r (k c)")
                        p2 = spool.tile([128, 2048], bf16, tag="p2")
                        nc.vector.tensor_mul(p2, pflat, pflat)
                        s01 = spool.tile([128, 1024], bf16, tag="s01")
                        nc.vector.tensor_add(s01, p2[:, 0:1024], p2[:, 1024:2048])
                        nc.vector.tensor_sub(
                            xb[:, T, 2:514], xb[:, T, 2:514], s01[:, 0:512])
                        nc.vector.tensor_sub(
                            xb[:, T, 2:514], xb[:, T, 2:514], s01[:, 512:1024])
                    # seam re-sync between adjacent tiles
                    nc.sync.dma_start(out=xb[0:2, 1:5, :], in_=xb[124:126, 0:4, :])
                    nc.sync.dma_start(out=xb[126:128, 0:4, :], in_=xb[2:4, 1:5, :])

            # write out (cast bf16 -> f32 during DMA)
            for im in range(NIMG):
                xb = xbufs[im]
                nc.gpsimd.dma_start(
                    out=out_ext[im, 0:496, :].rearrange("(tt p) c -> p tt c", p=124),
                    in_=xb[2:126, 0:4, 2:514])
                nc.gpsimd.dma_start(out=out_ext[im, 496:512, :],
                                    in_=xb[2:18, 4, 2:514])

    _split_multiwait_drains(nc)
    return nc


def _split_multiwait_drains(nc):
    """Walrus workaround: this compiler build only accepts one sem-wait per
    instruction; peel extras onto injected same-engine NoOps placed just
    before (engine streams run in program order, so semantics are equal)."""
    from concourse import mybir
    import bass_rust

    for f in nc.m.functions:
        for bb in f.blocks:
            idx = 0
            while idx < len(bb.instructions):
                inst = bb.instructions[idx]
                si = getattr(inst, "sync_info", None)
                if si is not None and si.on_wait and len(si.on_wait) > 1:
                    waits = list(si.on_wait)
                    upd = list(si.on_update) if si.on_update else []
                    for j, w in enumerate(waits[:-1]):
                        nop = mybir.InstNoOp(
                            name=f"{inst.name}-wsplit{j}", ins=[], outs=[])
                        nop.engine = inst.engine
                        nop.sync_info = bass_rust.SyncInfo(
                            on_wait=[w], on_update=[])
                        nc.register_instruction(nop, overwrite=True)
                        bb.instructions.insert(idx, nop)
                        idx += 1
                    inst.sync_info = bass_rust.SyncInfo(
                        on_wait=[waits[-1]], on_update=upd)
                idx += 1


def kernel(x, W, b, a):
    from concourse.bass_utils import run_bass_kernel_spmd

    x = np.asarray(x)
    Wf = np.asarray(W, dtype=np.float64)
    bf = np.asarray(b, dtype=np.float64)
    af = np.asarray(a, dtype=np.float64)

    mains, cls_, crs = [], [], []
    for t in range(NT):
        m, cl, cr = _build_bands(Wf[t])
        mains.append(m)
        cls_.append(cl)
        crs.append(cr)
    # transpose to [r, t, v, q, k, m] so the on-device DMA is contiguous
    bands = np.ascontiguousarray(
        np.stack(mains).transpose(4, 0, 1, 2, 3, 5)).astype(np.float32)
    corrl = np.ascontiguousarray(
        np.stack(cls_).transpose(3, 0, 1, 2, 4)).astype(np.float32)
    corrr = np.ascontiguousarray(
        np.stack(crs).transpose(3, 0, 1, 2, 4)).astype(np.float32)

    nc = _build_graph(Wf, bf, af)

    in_maps = []
    for core in range(NCORES):
        shard = x[core * NIMG:(core + 1) * NIMG, 0].astype(np.float32)
        in_maps.append({"x": np.ascontiguousarray(shard),
                        "bands": bands, "corrl": corrl, "corrr": corrr})
    res = run_bass_kernel_spmd(nc, in_maps, list(range(NCORES)))
    global LAST_RESULT
    LAST_RESULT = res
    out = np.concatenate([res.results[i]["out"] for i in range(NCORES)], axis=0)
    return out[:, None, :, :].astype(x.dtype)


LAST_RESULT = None
